# revision 5
# baseline (speedup 1.0000x reference)
"""Trainium2 Bass kernel for nn_Attention_7009386627377.

Multi-head attention (16 heads, d=64) over [4, 2048, 1024] hidden states,
sharded across 8 NeuronCores as (batch b = core//2, head-group g = core%2 of
8 heads). Each core computes its disjoint [2048, 512] output slice with no
collectives; the host reassembles [4, 2048, 16, 64].

v2: the exp (ScalarE ACTIVATE) stream is the bottleneck (~296us busy), so all
production work (hidden transposes, Q/K/V projections) is streamed into the
attention phase through scheduled work units that fill TensorE slack, with a
minimal prologue so the first ACTIVATE issues as early as possible.
16 single-pair groups (head-pair x s-quarter), pair-major order; PSUM layout:
4 banks score double-buffer + 1 ctx + 1 sums + 2 production. The attention
mask folds into the exp bias (per-partition bias column = (m-1)*100).
"""
import threading

import numpy as np

B = 4
S = 2048
HID = 1024
JC = 512          # per-core qkv columns = 8 heads x 64
D = 64
N_CORES = 8

_LOCK = threading.Lock()
_CACHE = {}


def _build(s=S):
    from contextlib import ExitStack

    from concourse import bacc, mybir
    import concourse.tile as tile
    from concourse.masks import make_identity

    F32 = mybir.dt.float32
    BF16 = mybir.dt.bfloat16
    EXP = mybir.ActivationFunctionType.Exp
    MUL = mybir.AluOpType.mult
    ADD = mybir.AluOpType.add

    nst = s // 128           # s-tiles
    nq = max(1, s // 512)    # 512-wide quarters of s
    qw = s // nq
    nkt = s // 128           # key tiles
    st_per_q = qw // 128

    nc = bacc.Bacc("TRN2", target_bir_lowering=False, debug=False,
                   enable_asserts=False)

    hid = nc.dram_tensor("hidden", [s, HID], F32, kind="ExternalInput").ap()
    msk = nc.dram_tensor("mask", [s, 1], F32, kind="ExternalInput").ap()
    wq_d = nc.dram_tensor("wq", [HID, JC], F32, kind="ExternalInput").ap()
    wk_d = nc.dram_tensor("wk", [HID, JC], F32, kind="ExternalInput").ap()
    wv_d = nc.dram_tensor("wv", [HID, JC], F32, kind="ExternalInput").ap()
    bq_d = nc.dram_tensor("bq", [JC, 1], F32, kind="ExternalInput").ap()
    bk_d = nc.dram_tensor("bk", [JC, 1], F32, kind="ExternalInput").ap()
    bv_d = nc.dram_tensor("bv", [1, JC], F32, kind="ExternalInput").ap()
    out_d = nc.dram_tensor("out", [s, JC], F32, kind="ExternalOutput").ap()

    with tile.TileContext(nc) as tc, ExitStack() as ctx:
        P = ctx.enter_context
        persist = P(tc.tile_pool(name="persist", bufs=1))
        dram_pool = P(tc.tile_pool(name="dram", bufs=1, space="DRAM"))
        hstage_pool = P(tc.tile_pool(name="hstage", bufs=3))
        hb_pool = P(tc.tile_pool(name="hb", bufs=2))
        wstage_pool = P(tc.tile_pool(name="wstage", bufs=2))
        pt_pool = P(tc.tile_pool(name="pt", bufs=6))
        ctx_sb_pool = P(tc.tile_pool(name="ctxsb", bufs=2))
        ssb_pool = P(tc.tile_pool(name="ssb", bufs=2))
        ot_pool = P(tc.tile_pool(name="ot", bufs=2))
        of_pool = P(tc.tile_pool(name="of", bufs=4))
        # PSUM: scores double-buffer 2x[128,1024]f32 (2 banks each) = 4,
        # ctx 1, sums 1, production 2 = 8 banks.
        ps_scores = P(tc.tile_pool(name="pssc", bufs=2, space="PSUM"))
        ps_ctx = P(tc.tile_pool(name="psctx", bufs=1, space="PSUM"))
        ps_sums = P(tc.tile_pool(name="pssums", bufs=1, space="PSUM"))
        ps_prod = P(tc.tile_pool(name="psprod", bufs=2, space="PSUM"))

        # ---- persistent SBUF ----
        ident_bf = persist.tile([128, 128], BF16, tag="ident_bf")
        make_identity(nc, ident_bf[:])
        ones32 = persist.tile([128, 32], BF16, tag="ones32")
        nc.vector.memset(ones32[:], 1.0)
        ones_row = persist.tile([1, 128], BF16, tag="ones_row")
        nc.vector.memset(ones_row[:], 1.0)

        # hT[:, hc*s + t*128 + j] = hidden[t*128 + j, hc*128 + part]
        hT = persist.tile([128, 8 * s], BF16, tag="hT")
        kT = [persist.tile([128, s], BF16, tag=f"kT{p}", name=f"kT{p}")
              for p in range(4)]
        qT = [persist.tile([128, s], BF16, tag=f"qT{p}", name=f"qT{p}")
              for p in range(4)]
        v_sb = [persist.tile([128, JC], BF16, tag=f"v{t}", name=f"v{t}")
                for t in range(nst)]
        w_sb = {w: persist.tile([128, 8 * JC], BF16, tag=f"w_{w}", name=f"w_{w}")
                for w in ("wk", "wq", "wv")}

        bq_sb = persist.tile([128, 4], F32, tag="bq_sb")
        bk_sb = persist.tile([128, 4], F32, tag="bk_sb")
        bv_st = persist.tile([1, JC], F32, tag="bv_st")
        bv_bf = persist.tile([1, JC], BF16, tag="bv_bf")
        mask_sb = persist.tile([128, nst], F32, tag="mask_sb")
        # exp bias column per k-tile: (m-1)*100 -> 0 where m=1, -100 (prob->0)
        # where m=0, matching (1-m)*f32_min in the reference softmax.
        ebias = persist.tile([128, nst], F32, tag="ebias")

        scratch = dram_pool.tile([4 * 144, s], BF16, tag="scratch")

        # ---- input DMAs (batched, strided APs; none on Scalar after the
        # weight stages; Scalar queue then carries only ACTIVATEs) ----
        hid_p = hid.rearrange("(t p) h -> p t h", p=128)      # [128,16,1024]
        hchunks = []
        for c in range(nst // 2):
            hs = hstage_pool.tile([128, 2, HID], F32, tag="hs", name=f"hs{c}")
            hchunks.append(hs)
        wstage = {}

        def dma_hchunk(c, eng):
            eng.dma_start(hchunks[c][:], hid_p[:, 2 * c:2 * c + 2, :])

        def dma_w(wname, wd, eng):
            st_t = wstage_pool.tile([128, 8, JC], F32, tag="wstage",
                                    name=f"wst_{wname}")
            wstage[wname] = st_t
            eng.dma_start(st_t[:], wd.rearrange("(c p) j -> p c j", p=128))

        dma_hchunk(0, nc.sync)
        dma_w("wk", wk_d, nc.scalar)
        dma_hchunk(1, nc.sync)
        dma_w("wq", wq_d, nc.gpsimd)
        dma_w("wv", wv_d, nc.scalar)
        nc.gpsimd.dma_start(bq_sb[:],
                            bq_d.rearrange("(c p) o -> p (c o)", p=128))
        nc.gpsimd.dma_start(bk_sb[:],
                            bk_d.rearrange("(c p) o -> p (c o)", p=128))
        nc.gpsimd.dma_start(bv_st[:], bv_d[:, :])
        nc.gpsimd.dma_start(mask_sb[:],
                            msk.rearrange("(t p) o -> p (t o)", p=128))
        for c in range(2, nst // 2):
            dma_hchunk(c, nc.sync)

        nc.vector.tensor_copy(bv_bf[:], bv_st[:])
        nc.vector.tensor_scalar(ebias[:], mask_sb[:], 100.0, -100.0, MUL, ADD)
        for wname in ("wk", "wq", "wv"):
            nc.vector.tensor_copy(
                w_sb[wname][:].rearrange("p (c j) -> p c j", c=8),
                wstage[wname][:])

        # ---- production units ----
        hT3 = hT[:].rearrange("p (c x) -> p c x", c=8)

        def u_transp(t):
            hb = hb_pool.tile([128, HID], BF16, tag="hb")
            nc.vector.tensor_copy(hb[:], hchunks[t // 2][:, t % 2, :])
            tp = ps_prod.tile([128, HID], BF16, tag="prod", name=f"tp{t}")
            for hc in range(8):
                nc.tensor.transpose(tp[:, hc * 128:(hc + 1) * 128],
                                    hb[:, hc * 128:(hc + 1) * 128],
                                    ident_bf[:])
            nc.vector.tensor_copy(hT3[:, :, t * 128:(t + 1) * 128],
                                  tp[:].rearrange("p (c x) -> p c x", c=8))

        def u_kqproj(wname, dst, b_sb, p, sq):
            pp = ps_prod.tile([128, qw], F32, tag="prod",
                              name=f"pp_{wname}{p}_{sq}")
            for hc in range(8):
                nc.tensor.matmul(
                    pp[:],
                    lhsT=w_sb[wname][:, hc * JC + p * 128:
                                     hc * JC + (p + 1) * 128],
                    rhs=hT[:, hc * s + sq * qw:hc * s + (sq + 1) * qw],
                    start=(hc == 0), stop=(hc == 7))
            nc.vector.tensor_scalar(dst[p][:, sq * qw:(sq + 1) * qw],
                                    pp[:], b_sb[:, p:p + 1], None, ADD)

        def u_vproj(t, half):
            vp = ps_prod.tile([128, 256], F32, tag="prod",
                              name=f"vp{t}_{half}")
            hw = slice(half * 256, half * 256 + 256)
            for hc in range(8):
                nc.tensor.matmul(
                    vp[:],
                    lhsT=hT[:, hc * s + t * 128:hc * s + (t + 1) * 128],
                    rhs=w_sb["wv"][:, hc * JC + half * 256:
                                   hc * JC + half * 256 + 256],
                    start=(hc == 0), stop=False)
            nc.tensor.matmul(vp[:], lhsT=ones_row[:], rhs=bv_bf[:, hw],
                             start=False, stop=True)
            nc.vector.tensor_copy(v_sb[t][:, hw], vp[:])

        kx = lambda p, sq: (lambda: u_kqproj("wk", kT, bk_sb, p, sq))
        qx = lambda p, q: (lambda: u_kqproj("wq", qT, bq_sb, p, q))
        vx = lambda t, h: (lambda: u_vproj(t, h))
        tx = lambda t: (lambda: u_transp(t))

        # group 1 per-step schedule (units issue at the END of step kt):
        # v_j <= step j (ctx(j) issues at step j+1); kproj(p0,sq) <= step
        # 4sq-1 (scores(4sq) issues at step 4sq); transp deps precede in-step.
        g1_sched = {
            0: [tx(4), vx(1, 0)], 1: [tx(5), vx(2, 0)], 2: [tx(6), vx(3, 0)],
            3: [tx(7), kx(0, 1)], 4: [tx(8), vx(4, 0)], 5: [tx(9), vx(5, 0)],
            6: [tx(10), vx(6, 0)], 7: [tx(11), kx(0, 2), vx(7, 0)],
            8: [tx(12), vx(8, 0)], 9: [tx(13), vx(9, 0)],
            10: [tx(14), vx(10, 0)], 11: [tx(15), kx(0, 3), vx(11, 0)],
            12: [vx(12, 0)], 13: [vx(13, 0)], 14: [vx(14, 0)],
            15: [vx(15, 0)],
        }

        # flat prefetch queue for groups 2..16 (REQ-gated + budget-pumped)
        T_PROJ, T_VHALF = 1750, 1000
        queue = []
        for q in range(1, 4):
            queue.append((T_PROJ, qx(0, q)))
        for sq in range(4):
            queue.append((T_PROJ, kx(1, sq)))
        for q in range(4):
            queue.append((T_PROJ, qx(1, q)))
        for t in range(nst):
            queue.append((T_VHALF, vx(t, 1)))
        for p in (2, 3):
            for sq in range(4):
                queue.append((T_PROJ, kx(p, sq)))
            for q in range(4):
                queue.append((T_PROJ, qx(p, q)))
        # prerequisite index into `queue` that must be drained before the
        # group's first scores matmul is issued (program order = semantics)
        REQ = {(0, 0): 0, (0, 1): 1, (0, 2): 2, (0, 3): 3,
               (1, 0): 8, (1, 1): 9, (1, 2): 10, (1, 3): 11,
               (2, 0): 32, (2, 1): 33, (2, 2): 34, (2, 3): 35,
               (3, 0): 40, (3, 1): 41, (3, 2): 42, (3, 3): 43}
        qidx = [0]

        def pump_until(idx):
            while qidx[0] < idx:
                _, fn = queue[qidx[0]]
                qidx[0] += 1
                fn()

        def pump_budget(budget_ns):
            spent = 0
            while qidx[0] < len(queue) and spent < budget_ns:
                cost, fn = queue[qidx[0]]
                qidx[0] += 1
                fn()
                spent += cost

        # ---- prologue production ----
        for t in range(4):
            u_transp(t)
        u_kqproj("wk", kT, bk_sb, 0, 0)
        u_kqproj("wq", qT, bq_sb, 0, 0)
        u_vproj(0, 0)

        # ---- attention groups: (pair p, quarter q), pair-major ----
        def run_group(p, q, sched):
            pump_until(REQ[(p, q)])
            qs = slice(q * qw, (q + 1) * qw)
            ctx_ps = ps_ctx.tile([128, qw], F32, tag="ctx", name=f"cx{p}_{q}")
            sums = ps_sums.tile([128, qw], F32, tag="sums", name=f"sm{p}_{q}")
            prev = [None]

            def scores_exp(kt):
                ks = slice(kt * 128, (kt + 1) * 128)
                sc = ps_scores.tile([128, 2 * qw], F32, tag="sc")
                nc.tensor.matmul(sc[:, 0:qw], lhsT=kT[p][0:64, ks],
                                 rhs=qT[p][0:64, qs], start=True, stop=True)
                nc.tensor.matmul(sc[:, qw:2 * qw], lhsT=kT[p][64:128, ks],
                                 rhs=qT[p][64:128, qs], start=True, stop=True)
                pt = pt_pool.tile([128, 2 * qw], BF16, tag="pt")
                nc.scalar.activation(pt[:], sc[:], EXP, scale=0.125,
                                     bias=ebias[:, kt:kt + 1])
                return pt

            def ctx_sums(kt, pt):
                nc.tensor.matmul(ctx_ps[0:64, :],
                                 lhsT=v_sb[kt][:, p * 128:p * 128 + 64],
                                 rhs=pt[:, 0:qw], start=(kt == 0),
                                 stop=(kt == nkt - 1), skip_group_check=True)
                nc.tensor.matmul(ctx_ps[64:128, :],
                                 lhsT=v_sb[kt][:, p * 128 + 64:p * 128 + 128],
                                 rhs=pt[:, qw:2 * qw], start=(kt == 0),
                                 stop=(kt == nkt - 1), skip_group_check=True)
                for i, pt_half in enumerate((pt[:, 0:qw], pt[:, qw:2 * qw])):
                    nc.tensor.matmul(
                        sums[32 * i:32 * (i + 1), :], lhsT=ones32[:],
                        rhs=pt_half, start=(kt == 0), stop=(kt == nkt - 1),
                        skip_group_check=True, tile_position=(0, 32 * i))

            for kt in range(nkt):
                pt = scores_exp(kt)
                if prev[0] is not None:
                    ctx_sums(*prev[0])
                prev[0] = (kt, pt)
                if sched is not None:
                    for fn in sched.get(kt, ()):
                        fn()
                else:
                    pump_budget(700)
            ctx_sums(*prev[0])

            # ---- close: evacuate, transpose via DRAM xbar, normalize ----
            base = 144 * p
            ctx_sb = ctx_sb_pool.tile([128, qw], BF16, tag="ctxsb")
            nc.vector.tensor_copy(ctx_sb[:], ctx_ps[:])
            nc.sync.dma_start(scratch[base:base + 128, qs], ctx_sb[:])
            # sums rows ride in partitions 0 and 32 (DVE is lane-locked);
            # DMA moves them to scratch rows 128/129 for the xbar transpose.
            ssb = ssb_pool.tile([128, qw], BF16, tag="ssb")
            nc.vector.tensor_copy(ssb[0:1, :], sums[0:1, :])
            nc.vector.tensor_copy(ssb[32:33, :], sums[32:33, :])
            nc.sync.dma_start(scratch[base + 128:base + 129, qs], ssb[0:1, :])
            nc.sync.dma_start(scratch[base + 129:base + 130, qs],
                              ssb[32:33, :])
            for b4 in range(qw // 128):
                sbg = q * st_per_q + b4
                ot = ot_pool.tile([128, 144], BF16, tag="ot")
                nc.sync.dma_start_transpose(
                    ot[:], scratch[base:base + 144,
                                   sbg * 128:(sbg + 1) * 128])
                rc = of_pool.tile([128, 2], F32, tag="rc",
                                  name=f"rc{p}_{sbg}")
                nc.vector.reciprocal(rc[:], ot[:, 128:130])
                of = of_pool.tile([128, 128], F32, tag="of")
                for h in range(2):
                    nc.vector.tensor_scalar(
                        of[:, h * D:(h + 1) * D],
                        ot[:, h * D:(h + 1) * D],
                        rc[:, h:h + 1], None, MUL)
                nc.sync.dma_start(
                    out_d[sbg * 128:(sbg + 1) * 128,
                          p * 128:(p + 1) * 128], of[:])

        for p in range(4):
            for q in range(4):
                run_group(p, q, g1_sched if (p, q) == (0, 0) else None)

    nc.compile()
    return nc


def _get_nc(s=S):
    with _LOCK:
        if s not in _CACHE:
            _CACHE[s] = _build(s)
        return _CACHE[s]


def _make_in_maps(inputs):
    hidden_states = np.asarray(inputs["hidden_states"], dtype=np.float32)
    attention_mask = np.asarray(inputs["attention_mask"], dtype=np.float32)
    Wq = np.asarray(inputs["Wq"], dtype=np.float32)
    Wk = np.asarray(inputs["Wk"], dtype=np.float32)
    Wv = np.asarray(inputs["Wv"], dtype=np.float32)
    bq = np.asarray(inputs["bq"], dtype=np.float32)
    bk = np.asarray(inputs["bk"], dtype=np.float32)
    bv = np.asarray(inputs["bv"], dtype=np.float32)

    in_maps = []
    for core in range(N_CORES):
        b, g = core // 2, core % 2
        js = slice(g * JC, (g + 1) * JC)
        in_maps.append({
            "hidden": np.ascontiguousarray(hidden_states[b]),
            "mask": np.ascontiguousarray(attention_mask[b].reshape(S, 1)),
            "wq": np.ascontiguousarray(Wq[:, js]),
            "wk": np.ascontiguousarray(Wk[:, js]),
            "wv": np.ascontiguousarray(Wv[:, js]),
            "bq": np.ascontiguousarray(bq[js].reshape(JC, 1)),
            "bk": np.ascontiguousarray(bk[js].reshape(JC, 1)),
            "bv": np.ascontiguousarray(bv[js].reshape(1, JC)),
        })
    return in_maps


def kernel(hidden_states, attention_mask, Wq, bq, Wk, bk, Wv, bv):
    from concourse.bass_utils import run_bass_kernel_spmd

    nc = _get_nc()
    in_maps = _make_in_maps(dict(
        hidden_states=hidden_states, attention_mask=attention_mask,
        Wq=Wq, bq=bq, Wk=Wk, bk=bk, Wv=Wv, bv=bv))

    res = run_bass_kernel_spmd(nc, in_maps, core_ids=list(range(N_CORES)))
    out = np.empty((B, S, 16, D), dtype=np.float32)
    for core in range(N_CORES):
        b, g = core // 2, core % 2
        out[b, :, g * 8:(g + 1) * 8, :] = \
            res.results[core]["out"].reshape(S, 8, D)
    return out


# revision 7
# speedup vs baseline: 1.0370x; 1.0370x over previous
"""Trainium2 Bass kernel for nn_Attention_7009386627377.

Multi-head attention (16 heads, d=64) over [4, 2048, 1024] hidden states,
sharded across 8 NeuronCores as (batch b = core//2, head-group g = core%2 of
8 heads). Each core computes its disjoint [2048, 512] output slice with no
collectives; the host reassembles [4, 2048, 16, 64].

v2: the exp (ScalarE ACTIVATE) stream is the bottleneck (~296us busy), so all
production work (hidden transposes, Q/K/V projections) is streamed into the
attention phase through scheduled work units that fill TensorE slack, with a
minimal prologue so the first ACTIVATE issues as early as possible.
16 single-pair groups (head-pair x s-quarter), pair-major order; PSUM layout:
4 banks score double-buffer + 1 ctx + 1 sums + 2 production. The attention
mask folds into the exp bias (per-partition bias column = (m-1)*100).
"""
import threading

import numpy as np

B = 4
S = 2048
HID = 1024
JC = 512          # per-core qkv columns = 8 heads x 64
D = 64
N_CORES = 8

_LOCK = threading.Lock()
_CACHE = {}


def _build(s=S):
    from contextlib import ExitStack

    from concourse import bacc, mybir
    import concourse.tile as tile
    from concourse.masks import make_identity

    F32 = mybir.dt.float32
    BF16 = mybir.dt.bfloat16
    EXP = mybir.ActivationFunctionType.Exp
    MUL = mybir.AluOpType.mult
    ADD = mybir.AluOpType.add

    nst = s // 128           # s-tiles
    nq = max(1, s // 512)    # 512-wide quarters of s
    qw = s // nq
    nkt = s // 128           # key tiles
    st_per_q = qw // 128

    nc = bacc.Bacc("TRN2", target_bir_lowering=False, debug=False,
                   enable_asserts=False)

    hid = nc.dram_tensor("hidden", [s, HID], F32, kind="ExternalInput").ap()
    msk = nc.dram_tensor("mask", [s, 1], F32, kind="ExternalInput").ap()
    wq_d = nc.dram_tensor("wq", [HID, JC], F32, kind="ExternalInput").ap()
    wk_d = nc.dram_tensor("wk", [HID, JC], F32, kind="ExternalInput").ap()
    wv_d = nc.dram_tensor("wv", [HID, JC], F32, kind="ExternalInput").ap()
    bq_d = nc.dram_tensor("bq", [JC, 1], F32, kind="ExternalInput").ap()
    bk_d = nc.dram_tensor("bk", [JC, 1], F32, kind="ExternalInput").ap()
    bv_d = nc.dram_tensor("bv", [1, JC], F32, kind="ExternalInput").ap()
    out_d = nc.dram_tensor("out", [s, JC], F32, kind="ExternalOutput").ap()

    with tile.TileContext(nc) as tc, ExitStack() as ctx:
        P = ctx.enter_context
        persist = P(tc.tile_pool(name="persist", bufs=1))
        dram_pool = P(tc.tile_pool(name="dram", bufs=1, space="DRAM"))
        hstage_pool = P(tc.tile_pool(name="hstage", bufs=3))
        hb_pool = P(tc.tile_pool(name="hb", bufs=2))
        wstage_pool = P(tc.tile_pool(name="wstage", bufs=2))
        pt_pool = P(tc.tile_pool(name="pt", bufs=7))
        ctx_sb_pool = P(tc.tile_pool(name="ctxsb", bufs=2))
        ssb_pool = P(tc.tile_pool(name="ssb", bufs=2))
        ot_pool = P(tc.tile_pool(name="ot", bufs=2))
        of_pool = P(tc.tile_pool(name="of", bufs=4))
        # PSUM: scores double-buffer 2x[128,1024]f32 (2 banks each) = 4,
        # ctx 1, sums 1, production 2 = 8 banks.
        ps_scores = P(tc.tile_pool(name="pssc", bufs=2, space="PSUM"))
        ps_ctx = P(tc.tile_pool(name="psctx", bufs=1, space="PSUM"))
        ps_sums = P(tc.tile_pool(name="pssums", bufs=1, space="PSUM"))
        ps_prod = P(tc.tile_pool(name="psprod", bufs=2, space="PSUM"))

        # ---- persistent SBUF ----
        ident_bf = persist.tile([128, 128], BF16, tag="ident_bf")
        make_identity(nc, ident_bf[:])
        ones32 = persist.tile([128, 32], BF16, tag="ones32")
        nc.vector.memset(ones32[:], 1.0)
        ones_row = persist.tile([1, 128], BF16, tag="ones_row")
        nc.vector.memset(ones_row[:], 1.0)

        # hT[:, hc*s + t*128 + j] = hidden[t*128 + j, hc*128 + part]
        hT = persist.tile([128, 8 * s], BF16, tag="hT")
        kT = [persist.tile([128, s], BF16, tag=f"kT{p}", name=f"kT{p}")
              for p in range(4)]
        qT = [persist.tile([128, s], BF16, tag=f"qT{p}", name=f"qT{p}")
              for p in range(4)]
        v_sb = [persist.tile([128, JC], BF16, tag=f"v{t}", name=f"v{t}")
                for t in range(nst)]
        w_sb = {w: persist.tile([128, 8 * JC], BF16, tag=f"w_{w}", name=f"w_{w}")
                for w in ("wk", "wq", "wv")}

        bq_sb = persist.tile([128, 4], F32, tag="bq_sb")
        bk_sb = persist.tile([128, 4], F32, tag="bk_sb")
        bv_st = persist.tile([1, JC], F32, tag="bv_st")
        bv_bf = persist.tile([1, JC], BF16, tag="bv_bf")
        mask_sb = persist.tile([128, nst], F32, tag="mask_sb")
        # exp bias column per k-tile: (m-1)*100 -> 0 where m=1, -100 (prob->0)
        # where m=0, matching (1-m)*f32_min in the reference softmax.
        ebias = persist.tile([128, nst], F32, tag="ebias")

        scratch = dram_pool.tile([4 * 144, s], BF16, tag="scratch")

        # ---- input DMAs (batched, strided APs; none on Scalar after the
        # weight stages; Scalar queue then carries only ACTIVATEs) ----
        hid_p = hid.rearrange("(t p) h -> p t h", p=128)      # [128,16,1024]
        hchunks = []
        for c in range(nst // 2):
            hs = hstage_pool.tile([128, 2, HID], F32, tag="hs", name=f"hs{c}")
            hchunks.append(hs)
        wstage = {}

        def dma_hchunk(c, eng):
            eng.dma_start(hchunks[c][:], hid_p[:, 2 * c:2 * c + 2, :])

        def dma_w(wname, wd, eng):
            st_t = wstage_pool.tile([128, 8, JC], F32, tag="wstage",
                                    name=f"wst_{wname}")
            wstage[wname] = st_t
            eng.dma_start(st_t[:], wd.rearrange("(c p) j -> p c j", p=128))

        dma_hchunk(0, nc.sync)
        dma_w("wk", wk_d, nc.scalar)
        dma_hchunk(1, nc.sync)
        dma_w("wq", wq_d, nc.scalar)
        dma_w("wv", wv_d, nc.scalar)
        nc.gpsimd.dma_start(bq_sb[:],
                            bq_d.rearrange("(c p) o -> p (c o)", p=128))
        nc.gpsimd.dma_start(bk_sb[:],
                            bk_d.rearrange("(c p) o -> p (c o)", p=128))
        nc.gpsimd.dma_start(bv_st[:], bv_d[:, :])
        nc.gpsimd.dma_start(mask_sb[:],
                            msk.rearrange("(t p) o -> p (t o)", p=128))
        for c in range(2, nst // 2):
            dma_hchunk(c, nc.sync)

        nc.vector.tensor_copy(bv_bf[:], bv_st[:])
        nc.vector.tensor_scalar(ebias[:], mask_sb[:], 100.0, -100.0, MUL, ADD)

        def cast_w(wname):
            nc.vector.tensor_copy(
                w_sb[wname][:].rearrange("p (c j) -> p c j", c=8),
                wstage[wname][:])

        # ---- production units ----
        hT3 = hT[:].rearrange("p (c x) -> p c x", c=8)

        def u_transp(t):
            hb = hb_pool.tile([128, HID], BF16, tag="hb")
            nc.vector.tensor_copy(hb[:], hchunks[t // 2][:, t % 2, :])
            tp = ps_prod.tile([128, HID], BF16, tag="prod", name=f"tp{t}")
            for hc in range(8):
                nc.tensor.transpose(tp[:, hc * 128:(hc + 1) * 128],
                                    hb[:, hc * 128:(hc + 1) * 128],
                                    ident_bf[:])
            nc.vector.tensor_copy(hT3[:, :, t * 128:(t + 1) * 128],
                                  tp[:].rearrange("p (c x) -> p c x", c=8))

        def u_kqproj(wname, dst, b_sb, p, sq):
            pp = ps_prod.tile([128, qw], F32, tag="prod",
                              name=f"pp_{wname}{p}_{sq}")
            for hc in range(8):
                nc.tensor.matmul(
                    pp[:],
                    lhsT=w_sb[wname][:, hc * JC + p * 128:
                                     hc * JC + (p + 1) * 128],
                    rhs=hT[:, hc * s + sq * qw:hc * s + (sq + 1) * qw],
                    start=(hc == 0), stop=(hc == 7))
            nc.vector.tensor_scalar(dst[p][:, sq * qw:(sq + 1) * qw],
                                    pp[:], b_sb[:, p:p + 1], None, ADD)

        def u_vproj(t):
            vp = ps_prod.tile([128, JC], F32, tag="prod", name=f"vp{t}")
            for hc in range(8):
                nc.tensor.matmul(
                    vp[:],
                    lhsT=hT[:, hc * s + t * 128:hc * s + (t + 1) * 128],
                    rhs=w_sb["wv"][:, hc * JC:hc * JC + JC],
                    start=(hc == 0), stop=False)
            nc.tensor.matmul(vp[:], lhsT=ones_row[:], rhs=bv_bf[:],
                             start=False, stop=True)
            nc.vector.tensor_copy(v_sb[t][:], vp[:])

        kx = lambda p, sq: (lambda: u_kqproj("wk", kT, bk_sb, p, sq))
        qx = lambda p, q: (lambda: u_kqproj("wq", qT, bq_sb, p, q))
        vx = lambda t: (lambda: u_vproj(t))
        tx = lambda t: (lambda: u_transp(t))

        # group 1 per-step schedule (units issue at the END of step kt):
        # v_j <= step j+1 (ctx(j) issues at step j+2); kproj(p0,sq) <= step
        # 4sq-1 (scores(4sq) issues at step 4sq); transp deps precede in-step.
        g1_sched = {
            0: [tx(4), vx(0)], 1: [tx(5), vx(1)], 2: [tx(6), vx(2)],
            3: [tx(7), kx(0, 1)], 4: [tx(8), vx(3)], 5: [tx(9), vx(4)],
            6: [tx(10), vx(5)], 7: [tx(11), kx(0, 2), vx(6)],
            8: [tx(12), vx(7)], 9: [tx(13), vx(8)],
            10: [tx(14), vx(9)], 11: [tx(15), kx(0, 3), vx(10)],
            12: [vx(11), vx(12)], 13: [vx(13), vx(14)], 14: [vx(15)],
            15: [],
        }

        # flat prefetch queue for groups 2..16 (REQ-gated + budget-pumped)
        T_PROJ = 1900
        queue = []
        for q in range(1, 4):
            queue.append((T_PROJ, qx(0, q)))
        for p in (1, 2, 3):
            for sq in range(4):
                queue.append((T_PROJ, kx(p, sq)))
            for q in range(4):
                queue.append((T_PROJ, qx(p, q)))
        # prerequisite index into `queue` that must be drained before the
        # group's first scores matmul is issued (program order = semantics)
        REQ = {(0, 0): 0, (0, 1): 1, (0, 2): 2, (0, 3): 3,
               (1, 0): 8, (1, 1): 9, (1, 2): 10, (1, 3): 11,
               (2, 0): 16, (2, 1): 17, (2, 2): 18, (2, 3): 19,
               (3, 0): 24, (3, 1): 25, (3, 2): 26, (3, 3): 27}
        qidx = [0]

        def pump_until(idx):
            while qidx[0] < idx:
                _, fn = queue[qidx[0]]
                qidx[0] += 1
                fn()

        def pump_budget(budget_ns):
            spent = 0
            while qidx[0] < len(queue) and spent < budget_ns:
                cost, fn = queue[qidx[0]]
                qidx[0] += 1
                fn()
                spent += cost

        # ---- prologue production (casts issued just before first use,
        # so the DVE FIFO never blocks on a later-arriving DMA) ----
        u_transp(0)
        u_transp(1)
        cast_w("wk")
        u_transp(2)
        u_transp(3)
        u_kqproj("wk", kT, bk_sb, 0, 0)
        cast_w("wq")
        u_kqproj("wq", qT, bq_sb, 0, 0)
        cast_w("wv")

        # ---- attention groups: (pair p, quarter q), pair-major ----
        def run_group(p, q, sched):
            pump_until(REQ[(p, q)])
            qs = slice(q * qw, (q + 1) * qw)
            ctx_ps = ps_ctx.tile([128, qw], F32, tag="ctx", name=f"cx{p}_{q}")
            sums = ps_sums.tile([128, qw], F32, tag="sums", name=f"sm{p}_{q}")
            prev = [None]

            def scores_exp(kt):
                ks = slice(kt * 128, (kt + 1) * 128)
                sc = ps_scores.tile([128, 2 * qw], F32, tag="sc")
                nc.tensor.matmul(sc[:, 0:qw], lhsT=kT[p][0:64, ks],
                                 rhs=qT[p][0:64, qs], start=True, stop=True)
                nc.tensor.matmul(sc[:, qw:2 * qw], lhsT=kT[p][64:128, ks],
                                 rhs=qT[p][64:128, qs], start=True, stop=True)
                pt = pt_pool.tile([128, 2 * qw], BF16, tag="pt")
                nc.scalar.activation(pt[:], sc[:], EXP, scale=0.125,
                                     bias=ebias[:, kt:kt + 1])
                return pt

            def ctx_mm(kt, pt):
                nc.tensor.matmul(ctx_ps[0:64, :],
                                 lhsT=v_sb[kt][:, p * 128:p * 128 + 64],
                                 rhs=pt[:, 0:qw], start=(kt == 0),
                                 stop=(kt == nkt - 1), skip_group_check=True)
                nc.tensor.matmul(ctx_ps[64:128, :],
                                 lhsT=v_sb[kt][:, p * 128 + 64:p * 128 + 128],
                                 rhs=pt[:, qw:2 * qw], start=(kt == 0),
                                 stop=(kt == nkt - 1), skip_group_check=True)

            def sums_pair(j, ptA, ptB):
                # one 4-up col-packed span covers k-tiles j and j+1; the
                # even/odd partials merge after the output transpose.
                for pos, pth in ((0, ptA[:, 0:qw]), (32, ptA[:, qw:2 * qw]),
                                 (64, ptB[:, 0:qw]), (96, ptB[:, qw:2 * qw])):
                    nc.tensor.matmul(
                        sums[pos:pos + 32, :], lhsT=ones32[:], rhs=pth,
                        start=(j == 0), stop=(j == nkt - 2),
                        skip_group_check=True, tile_position=(0, pos))

            pts = {}
            for kt in range(nkt):
                pts[kt] = scores_exp(kt)
                if kt >= 2:
                    ctx_mm(kt - 2, pts[kt - 2])
                if kt >= 3 and kt % 2 == 1:
                    sums_pair(kt - 3, pts[kt - 3], pts[kt - 2])
                if sched is not None:
                    for fn in sched.get(kt, ()):
                        fn()
                else:
                    pump_budget(900)
            ctx_mm(nkt - 2, pts[nkt - 2])
            ctx_mm(nkt - 1, pts[nkt - 1])
            sums_pair(nkt - 2, pts[nkt - 2], pts[nkt - 1])

            # ---- close: evacuate, transpose via DRAM xbar, normalize ----
            base = 144 * p
            ctx_sb = ctx_sb_pool.tile([128, qw], BF16, tag="ctxsb")
            nc.vector.tensor_copy(ctx_sb[:], ctx_ps[:])
            nc.sync.dma_start(scratch[base:base + 128, qs], ctx_sb[:])
            # sums rows ride in partitions 0 and 32 (DVE is lane-locked);
            # DMA moves them to scratch rows 128/129 for the xbar transpose.
            ssb = ssb_pool.tile([128, qw], BF16, tag="ssb")
            for i in range(4):
                nc.vector.tensor_copy(ssb[32 * i:32 * i + 1, :],
                                      sums[32 * i:32 * i + 1, :])
                nc.sync.dma_start(scratch[base + 128 + i:base + 129 + i, qs],
                                  ssb[32 * i:32 * i + 1, :])
            for b4 in range(qw // 128):
                sbg = q * st_per_q + b4
                ot = ot_pool.tile([128, 144], BF16, tag="ot")
                nc.sync.dma_start_transpose(
                    ot[:], scratch[base:base + 144,
                                   sbg * 128:(sbg + 1) * 128])
                rc = of_pool.tile([128, 4], F32, tag="rc",
                                  name=f"rc{p}_{sbg}")
                nc.vector.scalar_tensor_tensor(
                    rc[:, 2:4], ot[:, 128:130], 1.0, ot[:, 130:132],
                    MUL, ADD)
                nc.vector.reciprocal(rc[:, 0:2], rc[:, 2:4])
                of = of_pool.tile([128, 128], F32, tag="of")
                for h in range(2):
                    nc.vector.tensor_scalar(
                        of[:, h * D:(h + 1) * D],
                        ot[:, h * D:(h + 1) * D],
                        rc[:, h:h + 1], None, MUL)
                nc.sync.dma_start(
                    out_d[sbg * 128:(sbg + 1) * 128,
                          p * 128:(p + 1) * 128], of[:])

        for p in range(4):
            for q in range(4):
                run_group(p, q, g1_sched if (p, q) == (0, 0) else None)

    nc.compile()
    return nc


def _get_nc(s=S):
    with _LOCK:
        if s not in _CACHE:
            _CACHE[s] = _build(s)
        return _CACHE[s]


def _make_in_maps(inputs):
    hidden_states = np.asarray(inputs["hidden_states"], dtype=np.float32)
    attention_mask = np.asarray(inputs["attention_mask"], dtype=np.float32)
    Wq = np.asarray(inputs["Wq"], dtype=np.float32)
    Wk = np.asarray(inputs["Wk"], dtype=np.float32)
    Wv = np.asarray(inputs["Wv"], dtype=np.float32)
    bq = np.asarray(inputs["bq"], dtype=np.float32)
    bk = np.asarray(inputs["bk"], dtype=np.float32)
    bv = np.asarray(inputs["bv"], dtype=np.float32)

    in_maps = []
    for core in range(N_CORES):
        b, g = core // 2, core % 2
        js = slice(g * JC, (g + 1) * JC)
        in_maps.append({
            "hidden": np.ascontiguousarray(hidden_states[b]),
            "mask": np.ascontiguousarray(attention_mask[b].reshape(S, 1)),
            "wq": np.ascontiguousarray(Wq[:, js]),
            "wk": np.ascontiguousarray(Wk[:, js]),
            "wv": np.ascontiguousarray(Wv[:, js]),
            "bq": np.ascontiguousarray(bq[js].reshape(JC, 1)),
            "bk": np.ascontiguousarray(bk[js].reshape(JC, 1)),
            "bv": np.ascontiguousarray(bv[js].reshape(1, JC)),
        })
    return in_maps


def kernel(hidden_states, attention_mask, Wq, bq, Wk, bk, Wv, bv):
    from concourse.bass_utils import run_bass_kernel_spmd

    nc = _get_nc()
    in_maps = _make_in_maps(dict(
        hidden_states=hidden_states, attention_mask=attention_mask,
        Wq=Wq, bq=bq, Wk=Wk, bk=bk, Wv=Wv, bv=bv))

    res = run_bass_kernel_spmd(nc, in_maps, core_ids=list(range(N_CORES)))
    out = np.empty((B, S, 16, D), dtype=np.float32)
    for core in range(N_CORES):
        b, g = core // 2, core % 2
        out[b, :, g * 8:(g + 1) * 8, :] = \
            res.results[core]["out"].reshape(S, 8, D)
    return out


# revision 8
# speedup vs baseline: 1.0649x; 1.0269x over previous
"""Trainium2 Bass kernel for nn_Attention_7009386627377.

Multi-head attention (16 heads, d=64) over [4, 2048, 1024] hidden states,
sharded across 8 NeuronCores as (batch b = core//2, head-group g = core%2 of
8 heads). Each core computes its disjoint [2048, 512] output slice with no
collectives; the host reassembles [4, 2048, 16, 64].

v2: the exp (ScalarE ACTIVATE) stream is the bottleneck (~296us busy), so all
production work (hidden transposes, Q/K/V projections) is streamed into the
attention phase through scheduled work units that fill TensorE slack, with a
minimal prologue so the first ACTIVATE issues as early as possible.
16 single-pair groups (head-pair x s-quarter), pair-major order; PSUM layout:
4 banks score double-buffer + 1 ctx + 1 sums + 2 production. The attention
mask folds into the exp bias (per-partition bias column = (m-1)*100).
"""
import threading

import numpy as np

B = 4
S = 2048
HID = 1024
JC = 512          # per-core qkv columns = 8 heads x 64
D = 64
N_CORES = 8

_LOCK = threading.Lock()
_CACHE = {}


def _build(s=S):
    from contextlib import ExitStack

    from concourse import bacc, mybir
    import concourse.tile as tile
    from concourse.masks import make_identity

    F32 = mybir.dt.float32
    BF16 = mybir.dt.bfloat16
    EXP = mybir.ActivationFunctionType.Exp
    MUL = mybir.AluOpType.mult
    ADD = mybir.AluOpType.add

    nst = s // 128           # s-tiles
    nq = max(1, s // 512)    # 512-wide quarters of s
    qw = s // nq
    nkt = s // 128           # key tiles
    st_per_q = qw // 128

    nc = bacc.Bacc("TRN2", target_bir_lowering=False, debug=False,
                   enable_asserts=False)

    hid = nc.dram_tensor("hidden", [s, HID], F32, kind="ExternalInput").ap()
    msk = nc.dram_tensor("mask", [s, 1], F32, kind="ExternalInput").ap()
    wq_d = nc.dram_tensor("wq", [HID, JC], F32, kind="ExternalInput").ap()
    wk_d = nc.dram_tensor("wk", [HID, JC], F32, kind="ExternalInput").ap()
    wv_d = nc.dram_tensor("wv", [HID, JC], F32, kind="ExternalInput").ap()
    bq_d = nc.dram_tensor("bq", [JC, 1], F32, kind="ExternalInput").ap()
    bk_d = nc.dram_tensor("bk", [JC, 1], F32, kind="ExternalInput").ap()
    bv_d = nc.dram_tensor("bv", [1, JC], F32, kind="ExternalInput").ap()
    out_d = nc.dram_tensor("out", [s, JC], F32, kind="ExternalOutput").ap()

    with tile.TileContext(nc) as tc, ExitStack() as ctx:
        P = ctx.enter_context
        persist = P(tc.tile_pool(name="persist", bufs=1))
        dram_pool = P(tc.tile_pool(name="dram", bufs=1, space="DRAM"))
        hstage_pool = P(tc.tile_pool(name="hstage", bufs=3))
        hb_pool = P(tc.tile_pool(name="hb", bufs=2))
        wstage_pool = P(tc.tile_pool(name="wstage", bufs=2))
        pt_pool = P(tc.tile_pool(name="pt", bufs=7))
        ctx_sb_pool = P(tc.tile_pool(name="ctxsb", bufs=2))
        ssb_pool = P(tc.tile_pool(name="ssb", bufs=2))
        ot_pool = P(tc.tile_pool(name="ot", bufs=2))
        of_pool = P(tc.tile_pool(name="of", bufs=4))
        # PSUM: scores double-buffer 2x[128,1024]f32 (2 banks each) = 4,
        # ctx 1, sums 1, production 2 = 8 banks.
        ps_scores = P(tc.tile_pool(name="pssc", bufs=2, space="PSUM"))
        ps_ctx = P(tc.tile_pool(name="psctx", bufs=1, space="PSUM"))
        ps_sums = P(tc.tile_pool(name="pssums", bufs=1, space="PSUM"))
        ps_prod = P(tc.tile_pool(name="psprod", bufs=2, space="PSUM"))

        # ---- persistent SBUF ----
        ident_bf = persist.tile([128, 128], BF16, tag="ident_bf")
        make_identity(nc, ident_bf[:])
        ones32 = persist.tile([128, 32], BF16, tag="ones32")
        nc.vector.memset(ones32[:], 1.0)
        ones_row = persist.tile([1, 128], BF16, tag="ones_row")
        nc.vector.memset(ones_row[:], 1.0)

        # hT[:, hc*s + t*128 + j] = hidden[t*128 + j, hc*128 + part]
        hT = persist.tile([128, 8 * s], BF16, tag="hT")
        kT = [persist.tile([128, s], BF16, tag=f"kT{p}", name=f"kT{p}")
              for p in range(4)]
        qT = [persist.tile([128, s], BF16, tag=f"qT{p}", name=f"qT{p}")
              for p in range(4)]
        v_sb = [persist.tile([128, JC], BF16, tag=f"v{t}", name=f"v{t}")
                for t in range(nst)]
        w_sb = {w: persist.tile([128, 8 * JC], BF16, tag=f"w_{w}", name=f"w_{w}")
                for w in ("wk", "wq", "wv")}

        bq_sb = persist.tile([128, 4], F32, tag="bq_sb")
        bk_sb = persist.tile([128, 4], F32, tag="bk_sb")
        bv_st = persist.tile([1, JC], F32, tag="bv_st")
        bv_bf = persist.tile([1, JC], BF16, tag="bv_bf")
        mask_sb = persist.tile([128, nst], F32, tag="mask_sb")
        # exp bias column per k-tile: (m-1)*100 -> 0 where m=1, -100 (prob->0)
        # where m=0, matching (1-m)*f32_min in the reference softmax.
        ebias = persist.tile([128, nst], F32, tag="ebias")

        scratch = dram_pool.tile([4 * 144, s], BF16, tag="scratch")

        # ---- input DMAs (batched, strided APs; none on Scalar after the
        # weight stages; Scalar queue then carries only ACTIVATEs) ----
        hid_p = hid.rearrange("(t p) h -> p t h", p=128)      # [128,16,1024]
        hchunks = []
        for c in range(nst // 2):
            hs = hstage_pool.tile([128, 2, HID], F32, tag="hs", name=f"hs{c}")
            hchunks.append(hs)
        wstage = {}

        def dma_hchunk(c, eng):
            eng.dma_start(hchunks[c][:], hid_p[:, 2 * c:2 * c + 2, :])

        def dma_w(wname, wd, eng):
            st_t = wstage_pool.tile([128, 8, JC], F32, tag="wstage",
                                    name=f"wst_{wname}")
            wstage[wname] = st_t
            eng.dma_start(st_t[:], wd.rearrange("(c p) j -> p c j", p=128))

        dma_hchunk(0, nc.sync)
        dma_w("wk", wk_d, nc.scalar)
        dma_hchunk(1, nc.sync)
        dma_w("wq", wq_d, nc.scalar)
        nc.gpsimd.dma_start(bq_sb[:],
                            bq_d.rearrange("(c p) o -> p (c o)", p=128))
        nc.gpsimd.dma_start(bk_sb[:],
                            bk_d.rearrange("(c p) o -> p (c o)", p=128))
        nc.gpsimd.dma_start(bv_st[:], bv_d[:, :])
        nc.gpsimd.dma_start(mask_sb[:],
                            msk.rearrange("(t p) o -> p (t o)", p=128))

        nc.vector.tensor_copy(bv_bf[:], bv_st[:])
        nc.vector.tensor_scalar(ebias[:], mask_sb[:], 100.0, -100.0, MUL, ADD)

        def cast_w(wname):
            nc.vector.tensor_copy(
                w_sb[wname][:].rearrange("p (c j) -> p c j", c=8),
                wstage[wname][:])

        # ---- production units ----
        hT3 = hT[:].rearrange("p (c x) -> p c x", c=8)

        def u_transp(t):
            hb = hb_pool.tile([128, HID], BF16, tag="hb")
            nc.vector.tensor_copy(hb[:], hchunks[t // 2][:, t % 2, :])
            tp = ps_prod.tile([128, HID], BF16, tag="prod", name=f"tp{t}")
            for hc in range(8):
                nc.tensor.transpose(tp[:, hc * 128:(hc + 1) * 128],
                                    hb[:, hc * 128:(hc + 1) * 128],
                                    ident_bf[:])
            nc.vector.tensor_copy(hT3[:, :, t * 128:(t + 1) * 128],
                                  tp[:].rearrange("p (c x) -> p c x", c=8))

        def u_kqproj(wname, dst, b_sb, p, sq):
            pp = ps_prod.tile([128, qw], F32, tag="prod",
                              name=f"pp_{wname}{p}_{sq}")
            for hc in range(8):
                nc.tensor.matmul(
                    pp[:],
                    lhsT=w_sb[wname][:, hc * JC + p * 128:
                                     hc * JC + (p + 1) * 128],
                    rhs=hT[:, hc * s + sq * qw:hc * s + (sq + 1) * qw],
                    start=(hc == 0), stop=(hc == 7))
            nc.vector.tensor_scalar(dst[p][:, sq * qw:(sq + 1) * qw],
                                    pp[:], b_sb[:, p:p + 1], None, ADD)

        def u_vproj(t):
            vp = ps_prod.tile([128, JC], F32, tag="prod", name=f"vp{t}")
            for hc in range(8):
                nc.tensor.matmul(
                    vp[:],
                    lhsT=hT[:, hc * s + t * 128:hc * s + (t + 1) * 128],
                    rhs=w_sb["wv"][:, hc * JC:hc * JC + JC],
                    start=(hc == 0), stop=False)
            nc.tensor.matmul(vp[:], lhsT=ones_row[:], rhs=bv_bf[:],
                             start=False, stop=True)
            nc.vector.tensor_copy(v_sb[t][:], vp[:])

        kx = lambda p, sq: (lambda: u_kqproj("wk", kT, bk_sb, p, sq))
        qx = lambda p, q: (lambda: u_kqproj("wq", qT, bq_sb, p, q))
        vx = lambda t: (lambda: u_vproj(t))
        tx = lambda t: (lambda: u_transp(t))

        # group 1 per-step schedule (units issue at the END of step kt):
        # v_j <= step j+1 (ctx(j) issues at step j+2); kproj(p0,sq) <= step
        # 4sq-1 (scores(4sq) issues at step 4sq); transp deps precede in-step.
        g1_sched = {
            0: [tx(4), vx(0)], 1: [tx(5), vx(1)], 2: [tx(6), vx(2)],
            3: [tx(7), kx(0, 1)], 4: [tx(8), vx(3)], 5: [tx(9), vx(4)],
            6: [tx(10), vx(5)], 7: [tx(11), kx(0, 2), vx(6)],
            8: [tx(12), vx(7)], 9: [tx(13), vx(8)],
            10: [tx(14), vx(9)], 11: [tx(15), kx(0, 3), vx(10)],
            12: [vx(11), vx(12)], 13: [vx(13), vx(14)], 14: [vx(15)],
            15: [],
        }

        # flat prefetch queue for groups 2..16 (REQ-gated + budget-pumped)
        T_PROJ = 1900
        queue = []
        for q in range(1, 4):
            queue.append((T_PROJ, qx(0, q)))
        for p in (1, 2, 3):
            for sq in range(4):
                queue.append((T_PROJ, kx(p, sq)))
            for q in range(4):
                queue.append((T_PROJ, qx(p, q)))
        # prerequisite index into `queue` that must be drained before the
        # group's first scores matmul is issued (program order = semantics)
        REQ = {(0, 0): 0, (0, 1): 1, (0, 2): 2, (0, 3): 3,
               (1, 0): 8, (1, 1): 9, (1, 2): 10, (1, 3): 11,
               (2, 0): 16, (2, 1): 17, (2, 2): 18, (2, 3): 19,
               (3, 0): 24, (3, 1): 25, (3, 2): 26, (3, 3): 27}
        qidx = [0]

        def pump_until(idx):
            while qidx[0] < idx:
                _, fn = queue[qidx[0]]
                qidx[0] += 1
                fn()

        def pump_budget(budget_ns):
            spent = 0
            while qidx[0] < len(queue) and spent < budget_ns:
                cost, fn = queue[qidx[0]]
                qidx[0] += 1
                fn()
                spent += cost

        # ---- prologue production (casts issued just before first use,
        # so the DVE FIFO never blocks on a later-arriving DMA) ----
        u_transp(0)
        u_transp(1)
        cast_w("wk")
        u_transp(2)
        u_transp(3)
        u_kqproj("wk", kT, bk_sb, 0, 0)
        cast_w("wq")
        u_kqproj("wq", qT, bq_sb, 0, 0)
        dma_w("wv", wv_d, nc.scalar)
        for c in range(2, nst // 2):
            dma_hchunk(c, nc.sync)
        cast_w("wv")

        # ---- attention groups: (pair p, quarter q), pair-major ----
        def run_group(p, q, sched):
            pump_until(REQ[(p, q)])
            qs = slice(q * qw, (q + 1) * qw)
            ctx_ps = ps_ctx.tile([128, qw], F32, tag="ctx", name=f"cx{p}_{q}")
            sums = ps_sums.tile([128, qw], F32, tag="sums", name=f"sm{p}_{q}")
            prev = [None]

            def scores_exp(kt):
                ks = slice(kt * 128, (kt + 1) * 128)
                sc = ps_scores.tile([128, 2 * qw], F32, tag="sc")
                nc.tensor.matmul(sc[:, 0:qw], lhsT=kT[p][0:64, ks],
                                 rhs=qT[p][0:64, qs], start=True, stop=True)
                nc.tensor.matmul(sc[:, qw:2 * qw], lhsT=kT[p][64:128, ks],
                                 rhs=qT[p][64:128, qs], start=True, stop=True)
                pt = pt_pool.tile([128, 2 * qw], BF16, tag="pt")
                nc.scalar.activation(pt[:], sc[:], EXP, scale=0.125,
                                     bias=ebias[:, kt:kt + 1])
                return pt

            def ctx_mm(kt, pt):
                nc.tensor.matmul(ctx_ps[0:64, :],
                                 lhsT=v_sb[kt][:, p * 128:p * 128 + 64],
                                 rhs=pt[:, 0:qw], start=(kt == 0),
                                 stop=(kt == nkt - 1), skip_group_check=True)
                nc.tensor.matmul(ctx_ps[64:128, :],
                                 lhsT=v_sb[kt][:, p * 128 + 64:p * 128 + 128],
                                 rhs=pt[:, qw:2 * qw], start=(kt == 0),
                                 stop=(kt == nkt - 1), skip_group_check=True)

            def sums_pair(j, ptA, ptB):
                # one 4-up col-packed span covers k-tiles j and j+1; the
                # even/odd partials merge after the output transpose.
                for pos, pth in ((0, ptA[:, 0:qw]), (32, ptA[:, qw:2 * qw]),
                                 (64, ptB[:, 0:qw]), (96, ptB[:, qw:2 * qw])):
                    nc.tensor.matmul(
                        sums[pos:pos + 32, :], lhsT=ones32[:], rhs=pth,
                        start=(j == 0), stop=(j == nkt - 2),
                        skip_group_check=True, tile_position=(0, pos))

            pts = {}
            for kt in range(nkt):
                pts[kt] = scores_exp(kt)
                if kt >= 2:
                    ctx_mm(kt - 2, pts[kt - 2])
                if kt >= 3 and kt % 2 == 1:
                    sums_pair(kt - 3, pts[kt - 3], pts[kt - 2])
                if sched is not None:
                    for fn in sched.get(kt, ()):
                        fn()
                else:
                    pump_budget(250)
            ctx_mm(nkt - 2, pts[nkt - 2])
            ctx_mm(nkt - 1, pts[nkt - 1])
            sums_pair(nkt - 2, pts[nkt - 2], pts[nkt - 1])

            # ---- close: evacuate, transpose via DRAM xbar, normalize ----
            base = 144 * p
            ctx_sb = ctx_sb_pool.tile([128, qw], BF16, tag="ctxsb")
            nc.vector.tensor_copy(ctx_sb[:], ctx_ps[:])
            nc.sync.dma_start(scratch[base:base + 128, qs], ctx_sb[:])
            # sums rows ride in partitions 0 and 32 (DVE is lane-locked);
            # DMA moves them to scratch rows 128/129 for the xbar transpose.
            ssb = ssb_pool.tile([128, qw], BF16, tag="ssb")
            for i in range(4):
                nc.vector.tensor_copy(ssb[32 * i:32 * i + 1, :],
                                      sums[32 * i:32 * i + 1, :])
                nc.sync.dma_start(scratch[base + 128 + i:base + 129 + i, qs],
                                  ssb[32 * i:32 * i + 1, :])
            for b4 in range(qw // 128):
                sbg = q * st_per_q + b4
                ot = ot_pool.tile([128, 144], BF16, tag="ot")
                nc.sync.dma_start_transpose(
                    ot[:], scratch[base:base + 144,
                                   sbg * 128:(sbg + 1) * 128])
                rc = of_pool.tile([128, 4], F32, tag="rc",
                                  name=f"rc{p}_{sbg}")
                nc.vector.scalar_tensor_tensor(
                    rc[:, 2:4], ot[:, 128:130], 1.0, ot[:, 130:132],
                    MUL, ADD)
                nc.vector.reciprocal(rc[:, 0:2], rc[:, 2:4])
                of = of_pool.tile([128, 128], F32, tag="of")
                for h in range(2):
                    nc.vector.tensor_scalar(
                        of[:, h * D:(h + 1) * D],
                        ot[:, h * D:(h + 1) * D],
                        rc[:, h:h + 1], None, MUL)
                nc.gpsimd.dma_start(
                    out_d[sbg * 128:(sbg + 1) * 128,
                          p * 128:(p + 1) * 128], of[:])

        for p in range(4):
            for q in range(4):
                run_group(p, q, g1_sched if (p, q) == (0, 0) else None)

    nc.compile()
    return nc


def _get_nc(s=S):
    with _LOCK:
        if s not in _CACHE:
            _CACHE[s] = _build(s)
        return _CACHE[s]


def _make_in_maps(inputs):
    hidden_states = np.asarray(inputs["hidden_states"], dtype=np.float32)
    attention_mask = np.asarray(inputs["attention_mask"], dtype=np.float32)
    Wq = np.asarray(inputs["Wq"], dtype=np.float32)
    Wk = np.asarray(inputs["Wk"], dtype=np.float32)
    Wv = np.asarray(inputs["Wv"], dtype=np.float32)
    bq = np.asarray(inputs["bq"], dtype=np.float32)
    bk = np.asarray(inputs["bk"], dtype=np.float32)
    bv = np.asarray(inputs["bv"], dtype=np.float32)

    in_maps = []
    for core in range(N_CORES):
        b, g = core // 2, core % 2
        js = slice(g * JC, (g + 1) * JC)
        in_maps.append({
            "hidden": np.ascontiguousarray(hidden_states[b]),
            "mask": np.ascontiguousarray(attention_mask[b].reshape(S, 1)),
            "wq": np.ascontiguousarray(Wq[:, js]),
            "wk": np.ascontiguousarray(Wk[:, js]),
            "wv": np.ascontiguousarray(Wv[:, js]),
            "bq": np.ascontiguousarray(bq[js].reshape(JC, 1)),
            "bk": np.ascontiguousarray(bk[js].reshape(JC, 1)),
            "bv": np.ascontiguousarray(bv[js].reshape(1, JC)),
        })
    return in_maps


def kernel(hidden_states, attention_mask, Wq, bq, Wk, bk, Wv, bv):
    from concourse.bass_utils import run_bass_kernel_spmd

    nc = _get_nc()
    in_maps = _make_in_maps(dict(
        hidden_states=hidden_states, attention_mask=attention_mask,
        Wq=Wq, bq=bq, Wk=Wk, bk=bk, Wv=Wv, bv=bv))

    res = run_bass_kernel_spmd(nc, in_maps, core_ids=list(range(N_CORES)))
    out = np.empty((B, S, 16, D), dtype=np.float32)
    for core in range(N_CORES):
        b, g = core // 2, core % 2
        out[b, :, g * 8:(g + 1) * 8, :] = \
            res.results[core]["out"].reshape(S, 8, D)
    return out


# revision 10
# speedup vs baseline: 1.0846x; 1.0185x over previous
"""Trainium2 Bass kernel for nn_Attention_7009386627377.

Multi-head attention (16 heads, d=64) over [4, 2048, 1024] hidden states,
sharded across 8 NeuronCores as (batch b = core//2, head-group g = core%2 of
8 heads). Each core computes its disjoint [2048, 512] output slice with no
collectives; the host reassembles [4, 2048, 16, 64].

v2: the exp (ScalarE ACTIVATE) stream is the bottleneck (~296us busy), so all
production work (hidden transposes, Q/K/V projections) is streamed into the
attention phase through scheduled work units that fill TensorE slack, with a
minimal prologue so the first ACTIVATE issues as early as possible.
16 single-pair groups (head-pair x s-quarter), pair-major order; PSUM layout:
4 banks score double-buffer + 1 ctx + 1 sums + 2 production. The attention
mask folds into the exp bias (per-partition bias column = (m-1)*100).
"""
import threading

import numpy as np

B = 4
S = 2048
HID = 1024
JC = 512          # per-core qkv columns = 8 heads x 64
D = 64
N_CORES = 8

_LOCK = threading.Lock()
_CACHE = {}


def _build(s=S):
    from contextlib import ExitStack

    from concourse import bacc, mybir
    import concourse.tile as tile
    from concourse.masks import make_identity

    F32 = mybir.dt.float32
    BF16 = mybir.dt.bfloat16
    EXP = mybir.ActivationFunctionType.Exp
    MUL = mybir.AluOpType.mult
    ADD = mybir.AluOpType.add

    nst = s // 128           # s-tiles
    nq = max(1, s // 512)    # 512-wide quarters of s
    qw = s // nq
    nkt = s // 128           # key tiles
    st_per_q = qw // 128

    nc = bacc.Bacc("TRN2", target_bir_lowering=False, debug=False,
                   enable_asserts=False)

    hid = nc.dram_tensor("hidden", [s, HID], F32, kind="ExternalInput").ap()
    msk = nc.dram_tensor("mask", [s, 1], F32, kind="ExternalInput").ap()
    wq_d = nc.dram_tensor("wq", [HID, JC], F32, kind="ExternalInput").ap()
    wk_d = nc.dram_tensor("wk", [HID, JC], F32, kind="ExternalInput").ap()
    wv_d = nc.dram_tensor("wv", [HID, JC], F32, kind="ExternalInput").ap()
    bq_d = nc.dram_tensor("bq", [JC, 1], F32, kind="ExternalInput").ap()
    bk_d = nc.dram_tensor("bk", [JC, 1], F32, kind="ExternalInput").ap()
    bv_d = nc.dram_tensor("bv", [1, JC], F32, kind="ExternalInput").ap()
    out_d = nc.dram_tensor("out", [s, JC], F32, kind="ExternalOutput").ap()

    with tile.TileContext(nc) as tc, ExitStack() as ctx:
        P = ctx.enter_context
        persist = P(tc.tile_pool(name="persist", bufs=1))
        dram_pool = P(tc.tile_pool(name="dram", bufs=1, space="DRAM"))
        hstage_pool = P(tc.tile_pool(name="hstage", bufs=3))
        hb_pool = P(tc.tile_pool(name="hb", bufs=2))
        wstage_pool = P(tc.tile_pool(name="wstage", bufs=3))
        wvstage_pool = P(tc.tile_pool(name="wvstage", bufs=1))
        pt_pool = P(tc.tile_pool(name="pt", bufs=7))
        ctx_sb_pool = P(tc.tile_pool(name="ctxsb", bufs=2))
        ssb_pool = P(tc.tile_pool(name="ssb", bufs=2))
        ot_pool = P(tc.tile_pool(name="ot", bufs=2))
        of_pool = P(tc.tile_pool(name="of", bufs=4))
        # PSUM: scores double-buffer 2x[128,1024]f32 (2 banks each) = 4,
        # ctx 1, sums 1, production 2 = 8 banks.
        ps_scores = P(tc.tile_pool(name="pssc", bufs=2, space="PSUM"))
        ps_ctx = P(tc.tile_pool(name="psctx", bufs=1, space="PSUM"))
        ps_sums = P(tc.tile_pool(name="pssums", bufs=1, space="PSUM"))
        ps_prod = P(tc.tile_pool(name="psprod", bufs=2, space="PSUM"))

        # ---- persistent SBUF ----
        ident_bf = persist.tile([128, 128], BF16, tag="ident_bf")
        make_identity(nc, ident_bf[:])
        ones32 = persist.tile([128, 32], BF16, tag="ones32")
        nc.vector.memset(ones32[:], 1.0)
        ones_row = persist.tile([1, 128], BF16, tag="ones_row")
        nc.vector.memset(ones_row[:], 1.0)

        # hT[:, hc*s + t*128 + j] = hidden[t*128 + j, hc*128 + part]
        hT = persist.tile([128, 8 * s], BF16, tag="hT")
        kT = [persist.tile([128, s], BF16, tag=f"kT{p}", name=f"kT{p}")
              for p in range(4)]
        qT = [persist.tile([128, s], BF16, tag=f"qT{p}", name=f"qT{p}")
              for p in range(4)]
        v_sb = [persist.tile([128, JC], BF16, tag=f"v{t}", name=f"v{t}")
                for t in range(nst)]
        w_sb = {w: persist.tile([128, 8 * JC], BF16, tag=f"w_{w}", name=f"w_{w}")
                for w in ("wk", "wq", "wv")}

        bq_sb = persist.tile([128, 4], F32, tag="bq_sb")
        bk_sb = persist.tile([128, 4], F32, tag="bk_sb")
        bv_st = persist.tile([1, JC], F32, tag="bv_st")
        bv_bf = persist.tile([1, JC], BF16, tag="bv_bf")
        mask_sb = persist.tile([128, nst], F32, tag="mask_sb")
        # exp bias column per k-tile: (m-1)*100 -> 0 where m=1, -100 (prob->0)
        # where m=0, matching (1-m)*f32_min in the reference softmax.
        ebias = persist.tile([128, nst], F32, tag="ebias")

        scratch = dram_pool.tile([4 * 144, s], BF16, tag="scratch")

        # ---- input DMAs (batched, strided APs; none on Scalar after the
        # weight stages; Scalar queue then carries only ACTIVATEs) ----
        hid_p = hid.rearrange("(t p) h -> p t h", p=128)      # [128,16,1024]
        # chunk c covers s-tiles CH[c]..CH[c+1]-1
        CH = [0, 1, 2, 4, 6, 8, 10, 12, 14, 16]
        hchunks = []
        for c in range(len(CH) - 1):
            w = CH[c + 1] - CH[c]
            hs = hstage_pool.tile([128, w, HID], F32,
                                  tag="hsA" if w == 1 else "hsB",
                                  name=f"hs{c}")
            hchunks.append(hs)

        def chunk_of(t):
            for c in range(len(CH) - 1):
                if CH[c] <= t < CH[c + 1]:
                    return c, t - CH[c]
            raise AssertionError(t)

        def dma_hchunk(c, eng):
            eng.dma_start(hchunks[c][:], hid_p[:, CH[c]:CH[c + 1], :])

        wdr = {"wk": wk_d, "wq": wq_d}
        wstage = {}

        def dma_w_p(wname, p, eng):
            st_t = wstage_pool.tile([128, 8, 128], F32, tag="wstage",
                                    name=f"wst_{wname}{p}")
            wstage[(wname, p)] = st_t
            eng.dma_start(st_t[:], wdr[wname][:, p * 128:(p + 1) * 128]
                          .rearrange("(c pp) j -> pp c j", pp=128))

        def cast_w_p(wname, p):
            nc.vector.tensor_copy(
                w_sb[wname][:, p * 1024:(p + 1) * 1024]
                .rearrange("p (c j) -> p c j", c=8),
                wstage.pop((wname, p))[:])

        def dma_w(wname, wd, eng):
            st_t = wvstage_pool.tile([128, 8, JC], F32, tag="wstage",
                                     name=f"wst_{wname}")
            wstage[wname] = st_t
            eng.dma_start(st_t[:], wd.rearrange("(c p) j -> p c j", p=128))

        dma_hchunk(0, nc.sync)
        dma_w_p("wk", 0, nc.scalar)
        dma_hchunk(1, nc.sync)
        dma_w_p("wq", 0, nc.gpsimd)
        dma_hchunk(2, nc.sync)
        nc.gpsimd.dma_start(bq_sb[:],
                            bq_d.rearrange("(c p) o -> p (c o)", p=128))
        nc.gpsimd.dma_start(bk_sb[:],
                            bk_d.rearrange("(c p) o -> p (c o)", p=128))
        nc.gpsimd.dma_start(bv_st[:], bv_d[:, :])
        nc.gpsimd.dma_start(mask_sb[:],
                            msk.rearrange("(t p) o -> p (t o)", p=128))

        nc.vector.tensor_copy(bv_bf[:], bv_st[:])
        nc.vector.tensor_scalar(ebias[:], mask_sb[:], 100.0, -100.0, MUL, ADD)

        def cast_w(wname):
            nc.vector.tensor_copy(
                w_sb[wname][:].rearrange("p (c j) -> p c j", c=8),
                wstage.pop(wname)[:])

        # ---- production units ----
        hT3 = hT[:].rearrange("p (c x) -> p c x", c=8)

        def u_transp(t):
            hb = hb_pool.tile([128, HID], BF16, tag="hb")
            c, off = chunk_of(t)
            nc.vector.tensor_copy(hb[:], hchunks[c][:, off, :])
            tp = ps_prod.tile([128, HID], BF16, tag="prod", name=f"tp{t}")
            for hc in range(8):
                nc.tensor.transpose(tp[:, hc * 128:(hc + 1) * 128],
                                    hb[:, hc * 128:(hc + 1) * 128],
                                    ident_bf[:])
            nc.vector.tensor_copy(hT3[:, :, t * 128:(t + 1) * 128],
                                  tp[:].rearrange("p (c x) -> p c x", c=8))

        def u_kqproj(wname, dst, b_sb, p, sq):
            pp = ps_prod.tile([128, qw], F32, tag="prod",
                              name=f"pp_{wname}{p}_{sq}")
            for hc in range(8):
                nc.tensor.matmul(
                    pp[:],
                    lhsT=w_sb[wname][:, p * 1024 + hc * 128:
                                     p * 1024 + (hc + 1) * 128],
                    rhs=hT[:, hc * s + sq * qw:hc * s + (sq + 1) * qw],
                    start=(hc == 0), stop=(hc == 7))
            nc.vector.tensor_scalar(dst[p][:, sq * qw:(sq + 1) * qw],
                                    pp[:], b_sb[:, p:p + 1], None, ADD)

        def u_vproj(t):
            vp = ps_prod.tile([128, JC], F32, tag="prod", name=f"vp{t}")
            for hc in range(8):
                nc.tensor.matmul(
                    vp[:],
                    lhsT=hT[:, hc * s + t * 128:hc * s + (t + 1) * 128],
                    rhs=w_sb["wv"][:, hc * JC:hc * JC + JC],
                    start=(hc == 0), stop=False)
            nc.tensor.matmul(vp[:], lhsT=ones_row[:], rhs=bv_bf[:],
                             start=False, stop=True)
            nc.vector.tensor_copy(v_sb[t][:], vp[:])

        kx = lambda p, sq: (lambda: u_kqproj("wk", kT, bk_sb, p, sq))
        qx = lambda p, q: (lambda: u_kqproj("wq", qT, bq_sb, p, q))
        vx = lambda t: (lambda: u_vproj(t))
        tx = lambda t: (lambda: u_transp(t))

        # group 1 per-step schedule (units issue at the END of step kt):
        # v_j <= step j+1 (ctx(j) issues at step j+2); kproj(p0,sq) <= step
        # 4sq-1 (scores(4sq) issues at step 4sq); transp deps precede in-step.
        g1_sched = {
            0: [tx(4), vx(0)], 1: [tx(5), vx(1)], 2: [tx(6), vx(2)],
            3: [tx(7), kx(0, 1)], 4: [tx(8), vx(3)], 5: [tx(9), vx(4)],
            6: [tx(10), vx(5)], 7: [tx(11), kx(0, 2), vx(6)],
            8: [tx(12), vx(7)], 9: [tx(13), vx(8)],
            10: [tx(14), vx(9)], 11: [tx(15), kx(0, 3), vx(10)],
            12: [vx(11), vx(12)], 13: [vx(13), vx(14)], 14: [vx(15)],
            15: [],
        }

        # flat prefetch queue for groups 2..16 (REQ-gated + budget-pumped)
        T_PROJ = 1900
        queue = []
        for q in range(1, 4):
            queue.append((T_PROJ, qx(0, q)))
        for p in (1, 2, 3):
            queue.append((700, lambda p=p: cast_w_p("wk", p)))
            for sq in range(4):
                queue.append((T_PROJ, kx(p, sq)))
            queue.append((700, lambda p=p: cast_w_p("wq", p)))
            for q in range(4):
                queue.append((T_PROJ, qx(p, q)))
        # prerequisite index into `queue` that must be drained before the
        # group's first scores matmul is issued (program order = semantics)
        REQ = {(0, 0): 0, (0, 1): 1, (0, 2): 2, (0, 3): 3,
               (1, 0): 10, (1, 1): 11, (1, 2): 12, (1, 3): 13,
               (2, 0): 20, (2, 1): 21, (2, 2): 22, (2, 3): 23,
               (3, 0): 30, (3, 1): 31, (3, 2): 32, (3, 3): 33}
        qidx = [0]

        def pump_until(idx):
            while qidx[0] < idx:
                _, fn = queue[qidx[0]]
                qidx[0] += 1
                fn()

        def pump_budget(budget_ns):
            spent = 0
            while qidx[0] < len(queue) and spent < budget_ns:
                cost, fn = queue[qidx[0]]
                qidx[0] += 1
                fn()
                spent += cost

        # ---- prologue production (casts issued just before first use,
        # so the DVE FIFO never blocks on a later-arriving DMA) ----
        u_transp(0)
        u_transp(1)
        cast_w_p("wk", 0)
        u_transp(2)
        u_transp(3)
        u_kqproj("wk", kT, bk_sb, 0, 0)
        cast_w_p("wq", 0)
        u_kqproj("wq", qT, bq_sb, 0, 0)
        # stream the rest of the inputs: hidden alternates sync/gpsimd,
        # wv + later wk/wq column-chunks behind them
        dma_w("wv", wv_d, nc.scalar)
        for c in range(3, len(CH) - 1):
            dma_hchunk(c, nc.sync if c % 2 == 1 else nc.gpsimd)
        for p in range(1, 4):
            dma_w_p("wk", p, nc.scalar)
            dma_w_p("wq", p, nc.gpsimd)
        cast_w("wv")

        # ---- attention groups: (pair p, quarter q), pair-major ----
        def run_group(p, q, sched):
            pump_until(REQ[(p, q)])
            qs = slice(q * qw, (q + 1) * qw)
            ctx_ps = ps_ctx.tile([128, qw], F32, tag="ctx", name=f"cx{p}_{q}")
            sums = ps_sums.tile([128, qw], F32, tag="sums", name=f"sm{p}_{q}")
            prev = [None]

            def scores_exp(kt):
                ks = slice(kt * 128, (kt + 1) * 128)
                sc = ps_scores.tile([128, 2 * qw], F32, tag="sc")
                nc.tensor.matmul(sc[:, 0:qw], lhsT=kT[p][0:64, ks],
                                 rhs=qT[p][0:64, qs], start=True, stop=True)
                nc.tensor.matmul(sc[:, qw:2 * qw], lhsT=kT[p][64:128, ks],
                                 rhs=qT[p][64:128, qs], start=True, stop=True)
                pt = pt_pool.tile([128, 2 * qw], BF16, tag="pt")
                nc.scalar.activation(pt[:], sc[:], EXP, scale=0.125,
                                     bias=ebias[:, kt:kt + 1])
                return pt

            def ctx_mm(kt, pt):
                nc.tensor.matmul(ctx_ps[0:64, :],
                                 lhsT=v_sb[kt][:, p * 128:p * 128 + 64],
                                 rhs=pt[:, 0:qw], start=(kt == 0),
                                 stop=(kt == nkt - 1), skip_group_check=True)
                nc.tensor.matmul(ctx_ps[64:128, :],
                                 lhsT=v_sb[kt][:, p * 128 + 64:p * 128 + 128],
                                 rhs=pt[:, qw:2 * qw], start=(kt == 0),
                                 stop=(kt == nkt - 1), skip_group_check=True)

            def sums_pair(j, ptA, ptB):
                # one 4-up col-packed span covers k-tiles j and j+1; the
                # even/odd partials merge after the output transpose.
                for pos, pth in ((0, ptA[:, 0:qw]), (32, ptA[:, qw:2 * qw]),
                                 (64, ptB[:, 0:qw]), (96, ptB[:, qw:2 * qw])):
                    nc.tensor.matmul(
                        sums[pos:pos + 32, :], lhsT=ones32[:], rhs=pth,
                        start=(j == 0), stop=(j == nkt - 2),
                        skip_group_check=True, tile_position=(0, pos))

            pts = {}
            for kt in range(nkt):
                pts[kt] = scores_exp(kt)
                if kt >= 2:
                    ctx_mm(kt - 2, pts[kt - 2])
                if kt >= 3 and kt % 2 == 1:
                    sums_pair(kt - 3, pts[kt - 3], pts[kt - 2])
                if sched is not None:
                    for fn in sched.get(kt, ()):
                        fn()
                else:
                    pump_budget(250)
            ctx_mm(nkt - 2, pts[nkt - 2])
            ctx_mm(nkt - 1, pts[nkt - 1])
            sums_pair(nkt - 2, pts[nkt - 2], pts[nkt - 1])

            # ---- close: evacuate, transpose via DRAM xbar, normalize ----
            base = 144 * p
            ctx_sb = ctx_sb_pool.tile([128, qw], BF16, tag="ctxsb")
            nc.vector.tensor_copy(ctx_sb[:], ctx_ps[:])
            nc.sync.dma_start(scratch[base:base + 128, qs], ctx_sb[:])
            # sums rows ride in partitions 0 and 32 (DVE is lane-locked);
            # DMA moves them to scratch rows 128/129 for the xbar transpose.
            ssb = ssb_pool.tile([128, qw], BF16, tag="ssb")
            for i in range(4):
                nc.vector.tensor_copy(ssb[32 * i:32 * i + 1, :],
                                      sums[32 * i:32 * i + 1, :])
                nc.sync.dma_start(scratch[base + 128 + i:base + 129 + i, qs],
                                  ssb[32 * i:32 * i + 1, :])
            for b4 in range(qw // 128):
                sbg = q * st_per_q + b4
                ot = ot_pool.tile([128, 144], BF16, tag="ot")
                nc.sync.dma_start_transpose(
                    ot[:], scratch[base:base + 144,
                                   sbg * 128:(sbg + 1) * 128])
                rc = of_pool.tile([128, 4], F32, tag="rc",
                                  name=f"rc{p}_{sbg}")
                nc.vector.scalar_tensor_tensor(
                    rc[:, 2:4], ot[:, 128:130], 1.0, ot[:, 130:132],
                    MUL, ADD)
                nc.vector.reciprocal(rc[:, 0:2], rc[:, 2:4])
                of = of_pool.tile([128, 128], F32, tag="of")
                for h in range(2):
                    nc.vector.tensor_scalar(
                        of[:, h * D:(h + 1) * D],
                        ot[:, h * D:(h + 1) * D],
                        rc[:, h:h + 1], None, MUL)
                nc.sync.dma_start(
                    out_d[sbg * 128:(sbg + 1) * 128,
                          p * 128:(p + 1) * 128], of[:])

        for p in range(4):
            for q in range(4):
                run_group(p, q, g1_sched if (p, q) == (0, 0) else None)

    nc.compile()
    return nc


def _get_nc(s=S):
    with _LOCK:
        if s not in _CACHE:
            _CACHE[s] = _build(s)
        return _CACHE[s]


def _make_in_maps(inputs):
    hidden_states = np.asarray(inputs["hidden_states"], dtype=np.float32)
    attention_mask = np.asarray(inputs["attention_mask"], dtype=np.float32)
    Wq = np.asarray(inputs["Wq"], dtype=np.float32)
    Wk = np.asarray(inputs["Wk"], dtype=np.float32)
    Wv = np.asarray(inputs["Wv"], dtype=np.float32)
    bq = np.asarray(inputs["bq"], dtype=np.float32)
    bk = np.asarray(inputs["bk"], dtype=np.float32)
    bv = np.asarray(inputs["bv"], dtype=np.float32)

    in_maps = []
    for core in range(N_CORES):
        b, g = core // 2, core % 2
        js = slice(g * JC, (g + 1) * JC)
        in_maps.append({
            "hidden": np.ascontiguousarray(hidden_states[b]),
            "mask": np.ascontiguousarray(attention_mask[b].reshape(S, 1)),
            "wq": np.ascontiguousarray(Wq[:, js]),
            "wk": np.ascontiguousarray(Wk[:, js]),
            "wv": np.ascontiguousarray(Wv[:, js]),
            "bq": np.ascontiguousarray(bq[js].reshape(JC, 1)),
            "bk": np.ascontiguousarray(bk[js].reshape(JC, 1)),
            "bv": np.ascontiguousarray(bv[js].reshape(1, JC)),
        })
    return in_maps


def kernel(hidden_states, attention_mask, Wq, bq, Wk, bk, Wv, bv):
    from concourse.bass_utils import run_bass_kernel_spmd

    nc = _get_nc()
    in_maps = _make_in_maps(dict(
        hidden_states=hidden_states, attention_mask=attention_mask,
        Wq=Wq, bq=bq, Wk=Wk, bk=bk, Wv=Wv, bv=bv))

    res = run_bass_kernel_spmd(nc, in_maps, core_ids=list(range(N_CORES)))
    out = np.empty((B, S, 16, D), dtype=np.float32)
    for core in range(N_CORES):
        b, g = core // 2, core % 2
        out[b, :, g * 8:(g + 1) * 8, :] = \
            res.results[core]["out"].reshape(S, 8, D)
    return out
